# revision 18
# baseline (speedup 1.0000x reference)
"""DayAdapter Trainium2 kernel.

y[b] = softsign(x[b] @ W[day_ids[b]] + b[day_ids[b]])
  x: [64, 1024, 512] f32, W: [24, 512, 512] f32, b: [24, 512] f32,
  day_ids: [64] i64.

Strategy: data-parallel over batch (8 samples per NeuronCore, 8 cores),
computing the TRANSPOSED output yT[e, t] = sum_d W[d, e] x[t, d] + b[e]:

  - Output partitions are the feature dim e, so the per-day bias is a
    per-partition scalar that folds into a single ACT op
    (Identity(acc + bias)) straight out of PSUM — no broadcast tile.
  - All wire traffic is bf16 (x, W in; y out, upcast on host): 20 MiB
    per core instead of 40 MiB. Host pre-arranges every tensor into the
    exact SBUF layout, and samples are moved in PAIRS (2 MiB x / 1 MiB
    W+bias / 2 MiB y transfers, 12 DMAs per core) to ride the flat part
    of the DMA efficiency curve (~341+ GB/s at >=1 MiB).
  - Elementwise work is the real budget constraint (DVE f32 is only
    ~123 G elem/s; Pool ~92 G elem/s via STT; ACT ~154 G elem/s), so
    softsign is 3 passes over [128, 1024] tiles:
      ACT:  tt = acc + bias           (also drains the 2-bank PSUM pair)
      DVE:  rec = 1/(1 + |tt|)        (ONE fused 8-stage custom-DVE op:
                                       d = max(1+t, 1-t), BITWISE_NOT
                                       reciprocal seed + one Chebyshev-
                                       Newton pass, ~0.17% max rel err)
      Pool/DVE: out = tt*rec -> bf16  (multiply split between the two
                                       engines; Pool uses the STT form)
  - Output DMA rides ACT's hardware DGE queue; input loads ride SP's.

Per-core busy estimates: DMA ~59-66 us (bottleneck), PE ~55 us (bf16),
DVE ~43 us, Pool ~42 us, ACT ~37 us.
"""

import sys

if "/opt/trn_rl_repo" not in sys.path:
    sys.path.insert(0, "/opt/trn_rl_repo")

import numpy as np

import concourse.bacc as bacc
import concourse.mybir as mybir
import concourse.tile as tile
from concourse.bass_utils import run_bass_kernel_spmd

N_CORES = 8
B = 64
T = 1024
D = 512
S = B // N_CORES  # 8 samples per core
SP = 2  # samples per DMA pair
NPAIR = S // SP  # 4 pairs per core
P = 128
KB = D // P  # 4 contraction blocks
EB = D // P  # 4 output-feature blocks
TB = T // 512  # 2 psum-bank blocks of the t dim
WCOL = KB * D + EB  # W columns + bias columns packed per partition

# ---------------------------------------------------------------------------
# Custom fused DVE op: rec = 1/(1 + |t|) in one 8-stage pass.
#   d  = max(1+t, 1-t) = 1 + |t|          (3 stages)
#   y0 = bitcast(~d) * C0                 (2 stages: reciprocal seed)
#   y1 = y0 * (C1 - d*y0)                 (3 stages: Chebyshev-Newton)
# Uses the RECIPROCAL_APPROX_FAST constants; dropping its second Newton
# pass costs ~0.17% max rel err, far inside the accuracy budget.
# ---------------------------------------------------------------------------
from concourse import dve_ops
from concourse.dve_spec import (
    AluOp,
    Bin,
    One,
    Spec,
    Src0,
    lower,
    maxx,
    _has_src1,
)
from concourse.dve_uop import DveOpSpec


def _ref_softsign_recip(in0, in1, s0, s1, imm2):
    d = (1.0 + np.abs(in0)).astype(np.float32)
    not_d = (~d.view(np.int32)).view(np.float32)
    y0 = not_d * np.float32(s0)
    return y0 * (np.float32(s1) - d * y0)


def _register_softsign_recip():
    name = "SOFTSIGN_RECIP_1P_ANT"
    if name in dve_ops._SUB_OPCODE_FOR_NAME:
        for op in dve_ops.OPS:
            if op.name == name:
                return op
    _d = maxx(One + Src0, One - Src0)
    _nd = Bin(AluOp.BITWISE_NOT, _d, _d)
    _y0 = _nd * dve_ops.C0
    body = _y0 * (dve_ops.C1 - _d * _y0)
    spec = Spec(body=body, reference=_ref_softsign_recip)
    row = dve_ops._CUSTOM_DVE_ROW_BASE + len(dve_ops.OPS)
    dve_ops._SUB_OPCODE_FOR_NAME[name] = row
    shas = {}
    for ver in ("v3", "v4"):
        tmp = DveOpSpec(
            name=name, opcode=row, uops=lower(spec, ver=ver),
            rd1_en=_has_src1(spec),
        )
        shas[ver] = tmp.sha(ver)
    op = dve_ops.DveOp(name, spec, subdim=False, uops_sha=shas)
    dve_ops.OPS.append(op)
    dve_ops.CUSTOM_DVE_SPECS[name] = spec
    return op


SOFTSIGN_RECIP_1P = _register_softsign_recip()
_RECIP_C = dve_ops.RECIP_APPROX_FAST_CONSTS

_CACHE = {}

# test.py reads this for exec_time_ns after a traced run.
LAST_RESULTS = None
TRACE = False

# Of the 32 (sample, eb) groups per core, every DVE_MUL_MOD-th final
# multiply runs on DVE; the rest run on Pool (STT form).
DVE_MUL_MOD = 1


def _build(bench_reps=None, mode="full"):
    key = ("prog", bench_reps, DVE_MUL_MOD, mode)
    if key in _CACHE:
        return _CACHE[key]

    bf16 = mybir.dt.bfloat16
    f32 = mybir.dt.float32

    nc = bacc.Bacc("TRN2", debug=False, num_devices=N_CORES)

    # xT[q, p, j, k, t]: x for sample pair q, sample j of the pair
    # Wb[q, p, j, :]: per-partition [W(k=0..3, e=0..511), bias(eb=0..3)]
    xT = nc.dram_tensor("xT", [NPAIR, P, SP, KB, T], bf16, kind="ExternalInput").ap()
    Wb = nc.dram_tensor("Wb", [NPAIR, P, SP, WCOL], bf16, kind="ExternalInput").ap()
    y = nc.dram_tensor("y", [NPAIR, P, SP, EB, T], bf16, kind="ExternalOutput").ap()

    compute_like = mode not in ("full", "dma")
    in_bufs = NPAIR if compute_like else 2
    with tile.TileContext(nc) as tc:
        with (
            tc.tile_pool(name="xt", bufs=in_bufs) as xt_pool,
            tc.tile_pool(name="w", bufs=in_bufs) as w_pool,
            tc.tile_pool(name="tt", bufs=3) as tt_pool,
            tc.tile_pool(name="work", bufs=3) as work_pool,
            tc.tile_pool(name="out", bufs=2) as out_pool,
            tc.tile_pool(name="psum", bufs=3, space="PSUM") as psum_pool,
        ):
            import contextlib

            loop_cm = (
                tc.For_i(
                    0,
                    bench_reps,
                    1,
                    hint_engines=(
                        mybir.EngineType.PE,
                        mybir.EngineType.Activation,
                        mybir.EngineType.DVE,
                        mybir.EngineType.SP,
                    ),
                )
                if bench_reps
                else contextlib.nullcontext()
            )
            loaded = {}

            def load(q):
                xs = xt_pool.tile([P, SP, KB, T], bf16, tag="xs")
                nc.sync.dma_start(xs[:], xT[q])
                ws = w_pool.tile([P, SP, WCOL], bf16, tag="ws")
                nc.scalar.dma_start(ws[:], Wb[q])
                loaded[q] = (xs, ws)

            if compute_like:
                # loads hoisted out of the bench loop: engines re-read the
                # same SBUF tiles every iteration
                for q in range(NPAIR):
                    load(q)
                persist = dict(loaded)

            with loop_cm:
                if mode == "dma":
                    # loads + stores only, no compute (memset on Pool just
                    # marks the out tile written; it overlaps the DMAs)
                    for q in range(NPAIR):
                        load(q)
                        xs, ws = loaded.pop(q)
                        outs = out_pool.tile([P, SP, EB, T], bf16, tag="out")
                        nc.gpsimd.memset(outs[:], 0)
                        nc.scalar.dma_start(y[q], outs[:])
                elif compute_like:
                    loaded.update(persist)
                else:
                    load(0)
                    if NPAIR > 1:
                        load(1)
                # compute-stage subset for component benches: c1=PE only,
                # c2=+ACT, c3=+DVE, c4/compute/full=everything
                nstage = 4
                if mode.startswith("c") and mode[1:].isdigit():
                    nstage = int(mode[1:])
                g = 0  # running (sample, eb) group index for the mul split
                for q in range(NPAIR if mode != "dma" else 0):
                    xs, ws = loaded.pop(q)
                    outs = (
                        out_pool.tile([P, SP, EB, T], bf16, tag="out", name="outs")
                        if nstage >= 4
                        else None
                    )
                    for j in range(SP):
                        for eb in range(EB):
                            if (
                                mode == "full"
                                and q + 2 < NPAIR
                                and j == 0
                                and eb == 1
                            ):
                                load(q + 2)
                            acc = psum_pool.tile([P, T], f32, tag="acc")
                            for tb in range(TB):
                                for k in range(KB):
                                    nc.tensor.matmul(
                                        acc[:, tb * 512 : (tb + 1) * 512],
                                        ws[:, j, k * D + eb * P : k * D + (eb + 1) * P],
                                        xs[:, j, k, tb * 512 : (tb + 1) * 512],
                                        start=(k == 0),
                                        stop=(k == KB - 1),
                                    )
                            if nstage >= 2:
                                # ACT: tt = acc + bias (per-partition
                                # scalar), drains both PSUM banks in one op
                                tt = tt_pool.tile([P, T], bf16, tag="tt")
                                nc.scalar.add(
                                    tt[:], acc[:],
                                    ws[:, j, KB * D + eb : KB * D + eb + 1],
                                )
                            if nstage >= 3:
                                # DVE: rec = 1/(1 + |tt|), one fused pass
                                rec = work_pool.tile([P, T], bf16, tag="rec")
                                nc.vector._custom_dve(
                                    SOFTSIGN_RECIP_1P,
                                    out=rec[:],
                                    in0=tt[:],
                                    s0=_RECIP_C["s0"],
                                    s1=_RECIP_C["s1"],
                                )
                            if nstage >= 4:
                                # final multiply, split DVE/Pool for balance
                                on_dve = (
                                    DVE_MUL_MOD > 0 and g % DVE_MUL_MOD == 0
                                )
                                eng = nc.vector if on_dve else nc.gpsimd
                                eng.tensor_tensor(
                                    outs[:, j, eb, :], tt[:], rec[:],
                                    mybir.AluOpType.mult,
                                )
                            g += 1
                    if mode == "full":
                        nc.gpsimd.dma_start(y[q], outs[:])

    nc.compile()
    _CACHE[key] = nc
    return nc


def _prepare_in_maps(x, day_ids, W, b):
    import ml_dtypes

    bf16 = ml_dtypes.bfloat16

    x = np.asarray(x, dtype=np.float32)
    W = np.asarray(W, dtype=np.float32)
    b = np.asarray(b, dtype=np.float32)
    ids = np.asarray(day_ids).astype(np.int64)

    # Host-side prep into exact SBUF layouts (partition dim first):
    #   xdev[s, p, k, t]       = x[s, t, k*P + p]
    #   wdev[s, p, k*D + e]    = W[ids[s], k*P + p, e]
    #   wdev[s, p, KB*D + eb]  = b[ids[s], eb*P + p]
    xdev = np.ascontiguousarray(
        x.transpose(0, 2, 1).reshape(B, KB, P, T).transpose(0, 2, 1, 3)
    ).astype(bf16)
    wflat = (
        W[ids].reshape(B, KB, P, D).transpose(0, 2, 1, 3).reshape(B, P, KB * D)
    )
    bflat = b[ids].reshape(B, EB, P).transpose(0, 2, 1)
    wdev = np.ascontiguousarray(
        np.concatenate([wflat, bflat], axis=2)
    ).astype(bf16)

    in_maps = []
    for c in range(N_CORES):
        lo, hi = c * S, (c + 1) * S
        in_maps.append(
            {
                "xT": xdev[lo:hi].reshape(NPAIR, SP, P, KB, T).transpose(0, 2, 1, 3, 4),
                "Wb": wdev[lo:hi].reshape(NPAIR, SP, P, WCOL).transpose(0, 2, 1, 3),
            }
        )
    return in_maps


def kernel(x, day_ids, W, b):
    global LAST_RESULTS
    in_maps = _prepare_in_maps(x, day_ids, W, b)
    nc = _build()
    res = run_bass_kernel_spmd(
        nc, in_maps, core_ids=list(range(N_CORES)), trace=TRACE
    )
    LAST_RESULTS = res
    # y[q, p, j, eb, t] = y_sample[q*SP+j][t, eb*P + p]
    ydev = np.concatenate(
        [res.results[c]["y"] for c in range(N_CORES)], axis=0
    )  # [B/SP*? ...] -> per-core [NPAIR, P, SP, EB, T]
    ydev = ydev.reshape(N_CORES * NPAIR, P, SP, EB, T)
    out = ydev.transpose(0, 2, 4, 3, 1).reshape(B, T, D)
    return np.ascontiguousarray(out).astype(np.float32)
